# revision 16
# baseline (speedup 1.0000x reference)
"""TRN2 Bass kernel for nn_CausalSelfAttention_4054449128214.

The reference returns out_s + stop_gradient(out_full - out_s), whose forward
value is exactly out_full — plain dense causal self-attention. So the kernel
computes: qkv = x@W_attn+b_attn, per-head causal softmax attention, y@W_proj+b_proj.

Sharding (8 cores, no collectives):
  Megatron head-parallel. Cores 0-3 own head pairs (0,1)..(6,7); cores 4-7 own
  heads 8..11 (run twice for SPMD shape-uniformity, second copy's W_proj rows
  zeroed). Each core computes its heads' Q/K columns, V^T directly via matmul
  (lhsT = x chunk, rhs = W_v), attention, and a partial row-sliced output
  projection; the host sums the 8 partials (the Megatron row-parallel
  all-reduce) and transposes back.

Perf structure:
  - all inputs packed into ONE dram blob, consumption-ordered; per-cc segments
    (wqk|wv|x0) so each DMA unlocks a full contraction-chunk of K/Q/V^T work
    (HWDGE issue is a serial 625ns/DMA resource — DMA count is precious).
  - all matmul operands bf16 (full PE rate, half DMA bytes, 2x DVE rate);
    fp32 PSUM accumulation.
  - K/Q/V^T accumulation groups interleaved per-cc so PE starts on chunk 0.
  - attention (Act-exp-limited) interleaved with the next phase's matmuls:
    attn(qt0) x qkv(tt1), attn(qt1) x proj(tt0).
  - bias algebra: b_k is softmax-invariant (dropped); b_v/b_proj fold into a
    host-side constant column vector (softmax rows sum to 1); only b_q in-kernel.
"""

import numpy as np
import ml_dtypes

import concourse.bacc as bacc
import concourse.mybir as mybir
import concourse.tile as tile
from concourse.bass_utils import run_bass_kernel_spmd

F32 = mybir.dt.float32
BF16 = mybir.dt.bfloat16

T = 1024          # sequence length
C = 768           # channels
NH = 12           # heads
HS = 64           # head size
NCORES = 8
TT = 512          # t-tile (matmul moving free dim)
NT = T // TT      # 2
NCC = C // 128    # 6 contraction chunks
NKC = T // 128    # 8 key chunks
SCALE = 1.0 / 8.0  # 1/sqrt(HS)

NDUMMY = 400              # PE p-state warmup matmuls (run during initial DMA wait)
SEG = 896                 # per-cc blob segment: wqk(256) | wv(128) | x0(512)
X1OFF = NCC * SEG         # 5376
WPOFF = X1OFF + NCC * TT  # 8448
BLOBW = WPOFF + C         # 9216

# core -> (head0, head1); cores 4-7 duplicate their head (2nd W_proj slice zeroed)
HEAD_MAP = [(0, 1), (2, 3), (4, 5), (6, 7), (8, 8), (9, 9), (10, 10), (11, 11)]

_CACHE: dict = {}


def _build_program():
    nc = bacc.Bacc("TRN2", target_bir_lowering=False, debug=False,
                   num_devices=NCORES)
    blob = nc.dram_tensor("blob", [128, BLOBW], BF16, kind="ExternalInput").ap()
    bq = nc.dram_tensor("bq", [128, 1], F32, kind="ExternalInput").ap()
    outT = nc.dram_tensor("outT", [C, T], BF16, kind="ExternalOutput").ap()

    with tile.TileContext(nc) as tc:
        with (
            tc.tile_pool(name="const", bufs=1) as cp,
            tc.tile_pool(name="e", bufs=8) as ep,
            tc.tile_pool(name="rb", bufs=4) as rbp,
            tc.tile_pool(name="pmm", bufs=2, space="PSUM") as pmm,
            tc.tile_pool(name="pst", bufs=2, space="PSUM") as pst,
            tc.tile_pool(name="pov", bufs=2, space="PSUM") as pov,
            tc.tile_pool(name="pvt", bufs=2, space="PSUM") as pvt,
        ):
            bsb = cp.tile([128, WPOFF], BF16, tag="bsb")
            wpt = cp.tile([64, 2 * C], BF16, tag="wpt")
            for cc in range(NCC):
                nc.sync.dma_start(out=bsb[:, cc * SEG:(cc + 1) * SEG],
                                  in_=blob[:, cc * SEG:(cc + 1) * SEG])
            nc.sync.dma_start(out=bsb[:, X1OFF:X1OFF + 3 * TT],
                              in_=blob[:, X1OFF:X1OFF + 3 * TT])
            nc.sync.dma_start(out=bsb[:, X1OFF + 3 * TT:WPOFF],
                              in_=blob[:, X1OFF + 3 * TT:WPOFF])
            nc.sync.dma_start(
                out=wpt[:].rearrange("p (h e) -> p h e", h=2),
                in_=blob[:, WPOFF:BLOBW].rearrange("(h p) e -> p h e", p=64))
            bq_sb = cp.tile([128, 1], F32, tag="bq")
            nc.gpsimd.dma_start(out=bq_sb[:], in_=bq)

            wqk = [bsb[:, cc * SEG:cc * SEG + 256] for cc in range(NCC)]
            wv = [bsb[:, cc * SEG + 256:cc * SEG + 384] for cc in range(NCC)]
            xts = [[bsb[:, cc * SEG + 384:(cc + 1) * SEG],
                    bsb[:, X1OFF + cc * TT:X1OFF + (cc + 1) * TT]]
                   for cc in range(NCC)]
            wps = [wpt[:, hi * C:(hi + 1) * C] for hi in range(2)]

            # V^T tiles: [128 keys, 65*2] with a ones column at 64 and 129
            vaug = [cp.tile([128, 130], BF16, tag=f"va{kc}", name=f"va{kc}")
                    for kc in range(NKC)]
            for kc in range(NKC):
                nc.vector.memset(vaug[kc][:, 64:130:65], 1.0)
            ones64 = cp.tile([1, 64], BF16, tag="ones64")
            nc.vector.memset(ones64[:], 1.0)
            # PE warmup: tiny matmuls during the initial DMA wait keep the
            # tensor engine's p-state ramping so real work runs at full clock
            pdum = pvt.tile([1, 1], F32, tag="pt", name="pdum")
            for _ in range(NDUMMY):
                nc.tensor.matmul(pdum[:], ones64[0:1, 0:1], ones64[0:1, 0:1],
                                 start=True, stop=True)

            # static causal masks for the DVE half of the mask work
            masks = []
            for kcr in range(4):
                m = cp.tile([128, TT], BF16, tag=f"mask{kcr}", name=f"mask{kcr}")
                nc.vector.memset(m[:], 1.0)
                nc.gpsimd.affine_select(
                    m[:], m[:], pattern=[[1, TT]],
                    compare_op=mybir.AluOpType.is_ge, fill=0.0,
                    base=-128 * kcr, channel_multiplier=-1)
                masks.append(m)

            qT = [None] * NT   # [128=(2h x 64hs), TT] bf16
            kT = [None] * NT
            yT = [[None] * NT for _ in range(2)]
            st8 = {}           # per-tt K/Q/V01 psum state

            def vcopy(kc, pt):
                nc.vector.tensor_copy(vaug[kc][:, 0:64], pt[:, 0:64])
                nc.vector.tensor_copy(vaug[kc][:, 65:129], pt[:, 64:128])

            def u_cc(tt, cc):
                # one contraction chunk of the K/Q/V0/V1 accumulations
                if cc == 0:
                    st8[tt] = (pmm.tile([128, TT], F32, tag="mm", name=f"psK{tt}"),
                               pmm.tile([128, TT], F32, tag="mm", name=f"psQ{tt}"),
                               pvt.tile([128, 128], F32, tag="pt", name=f"ptA{tt}"),
                               pvt.tile([128, 128], F32, tag="pt", name=f"ptB{tt}"))
                psK, psQ, ptA, ptB = st8[tt]
                st, sp = cc == 0, cc == NCC - 1
                nc.tensor.matmul(psK[:], wqk[cc][:, 128:256], xts[cc][tt],
                                 start=st, stop=sp)
                nc.tensor.matmul(psQ[:], wqk[cc][:, 0:128], xts[cc][tt],
                                 start=st, stop=sp)
                nc.tensor.matmul(ptA[:], xts[cc][tt][:, 0:128], wv[cc],
                                 start=st, stop=sp)
                nc.tensor.matmul(ptB[:], xts[cc][tt][:, 128:256], wv[cc],
                                 start=st, stop=sp)

            def u_fin(tt):
                psK, psQ, ptA, ptB = st8[tt]
                kT[tt] = cp.tile([128, TT], BF16, tag=f"k{tt}", name=f"k{tt}")
                nc.scalar.activation(
                    kT[tt][:], psK[:], mybir.ActivationFunctionType.Copy)
                qT[tt] = cp.tile([128, TT], BF16, tag=f"q{tt}", name=f"q{tt}")
                nc.vector.tensor_scalar_add(qT[tt][:], psQ[:], bq_sb[:, 0:1])
                vcopy(tt * 4 + 0, ptA)
                vcopy(tt * 4 + 1, ptB)

            def u_v23(tt, j):  # j in (2, 3)
                pt = pvt.tile([128, 128], F32, tag="pt")
                for cc in range(NCC):
                    nc.tensor.matmul(pt[:], xts[cc][tt][:, j * 128:(j + 1) * 128],
                                     wv[cc], start=(cc == 0), stop=(cc == NCC - 1))
                vcopy(tt * 4 + j, pt)

            def emit_attn(qt, fillers):
                fit = iter(fillers)
                # masked (diagonal) chunks first so the last chunk's chain is
                # QK->exp->PV with no mask step; order is math-irrelevant (sum)
                kcs = list(range(qt * 4, qt * 4 + 4)) + list(range(0, qt * 4))
                for hi in range(2):
                    po = pov.tile([65, TT], F32, tag="po")
                    for i, kc in enumerate(kcs):
                        last = i == len(kcs) - 1
                        ktile = kT[kc // 4]
                        kcol = (kc % 4) * 128
                        ps = pst.tile([128, TT], F32, tag="st")
                        nc.tensor.matmul(
                            ps[:], ktile[hi * 64:(hi + 1) * 64, kcol:kcol + 128],
                            qT[qt][hi * 64:(hi + 1) * 64, :],
                            start=True, stop=True)
                        e = ep.tile([128, TT], BF16, tag="e")
                        nc.scalar.activation(
                            e[:], ps[:], mybir.ActivationFunctionType.Exp,
                            scale=SCALE)
                        kcr = kc - qt * 4
                        if kcr >= 0:  # diagonal chunk: zero where tk > tq
                            if kcr % 2 == 0:
                                nc.gpsimd.affine_select(
                                    e[:], e[:], pattern=[[1, TT]],
                                    compare_op=mybir.AluOpType.is_ge, fill=0.0,
                                    base=-128 * kcr, channel_multiplier=-1)
                            else:
                                nc.vector.tensor_mul(e[:], e[:], masks[kcr][:])
                        va = vaug[kc][:, hi * 65:(hi + 1) * 65]
                        if not last:
                            nc.tensor.matmul(po[:], va, e[:],
                                             start=(i == 0), stop=False)
                        else:
                            # split the last PV by query halves so the two
                            # normalization chains pipeline
                            nc.tensor.matmul(po[:, 0:TT // 2], va,
                                             e[:, 0:TT // 2],
                                             start=False, stop=False)
                            nc.tensor.matmul(po[:, TT // 2:TT], va,
                                             e[:, TT // 2:TT],
                                             start=False, stop=True)
                        f = next(fit, None)
                        if f is not None:
                            f()
                    yT[hi][qt] = cp.tile([64, TT], BF16, tag=f"y{hi}_{qt}",
                                         name=f"y{hi}_{qt}")
                    for c0 in (0, TT // 2):
                        c1 = c0 + TT // 2
                        rb = rbp.tile([1, TT // 2], BF16, tag="rb")
                        with nc.allow_low_precision(
                                reason="bf16 1/s is a uniform 0.4% softmax-"
                                       "scale wobble, within the 2e-2 gate"):
                            nc.vector.reciprocal(rb[0:1, :], po[64:65, c0:c1])
                        rbc = rbp.tile([64, TT // 2], BF16, tag="rbc")
                        nc.gpsimd.partition_broadcast(rbc[:], rb[0:1, :])
                        nc.vector.tensor_mul(yT[hi][qt][:, c0:c1],
                                             po[0:64, c0:c1], rbc[:])
                for f in fit:
                    f()

            def u_et(tt, et, ost, col):
                # one output-projection column group into ost[:, col*TT:...]
                pm = pmm.tile([128, TT], F32, tag="mm")
                for hi in range(2):
                    nc.tensor.matmul(
                        pm[:], wps[hi][:, et * 128:(et + 1) * 128],
                        yT[hi][tt][:], start=(hi == 0), stop=(hi == 1))
                dst = ost[:, col * TT:(col + 1) * TT]
                if et % 3 == 0:
                    nc.scalar.activation(
                        dst, pm[:], mybir.ActivationFunctionType.Copy)
                else:
                    nc.vector.tensor_copy(dst, pm[:])

            def u_store(tt, et0, ngrp, ost):
                nc.sync.dma_start(
                    out=outT[et0 * 128:(et0 + ngrp) * 128,
                             tt * TT:(tt + 1) * TT]
                    .rearrange("(g p) t -> p g t", p=128),
                    in_=ost[:].rearrange("p (g t) -> p g t", g=ngrp))

            def proj_units(tt, groups):
                units = []
                for et0, ngrp in groups:
                    ost = cp.tile([128, ngrp * TT], BF16,
                                  tag=f"ost{tt}_{et0}", name=f"ost{tt}_{et0}")
                    for i in range(ngrp):
                        units.append(lambda tt=tt, et=et0 + i, ost=ost, col=i:
                                     u_et(tt, et, ost, col))
                    units.append(lambda tt=tt, et0=et0, ngrp=ngrp, ost=ost:
                                 u_store(tt, et0, ngrp, ost))
                return units

            # --- split-phase proj for tt1: hi0 matmuls prelaunch into idle
            # --- psum banks during attn(1) hi1; hi1 matmuls+copies after
            pm1 = {}

            def u_p1h0(et, pool):
                pm1[et] = pool.tile([128, TT], F32, tag="mm" if pool is pmm
                                    else ("pt" if pool is pvt else "st"),
                                    name=f"pm1_{et}")
                nc.tensor.matmul(pm1[et][:], wps[0][:, et * 128:(et + 1) * 128],
                                 yT[0][1][:], start=True, stop=False)

            def u_p1h1(et, ost, col, split):
                nc.tensor.matmul(pm1[et][:], wps[1][:, et * 128:(et + 1) * 128],
                                 yT[1][1][:], start=False, stop=True)
                dst = ost[:, col * TT:(col + 1) * TT]
                if split:  # halves on both engines so the last copy overlaps
                    nc.scalar.activation(dst[:, 0:TT // 2], pm1[et][:, 0:TT // 2],
                                         mybir.ActivationFunctionType.Copy)
                    nc.vector.tensor_copy(dst[:, TT // 2:TT],
                                          pm1[et][:, TT // 2:TT])
                elif et % 2 == 0:
                    nc.scalar.activation(dst, pm1[et][:],
                                         mybir.ActivationFunctionType.Copy)
                else:
                    nc.vector.tensor_copy(dst, pm1[et][:])

            # ---- phase C: qkv+vaug for tt0, DMA-paced per-cc ----
            for cc in range(NCC):
                u_cc(0, cc)
            u_fin(0)
            u_v23(0, 2)
            u_v23(0, 3)
            # ---- phase D: attn(qt0) interleaved with qkv+vaug(tt1) ----
            emit_attn(0, [lambda cc=cc: u_cc(1, cc) for cc in range(NCC)]
                      + [lambda: u_fin(1), lambda: u_v23(1, 2)])
            u_v23(1, 3)
            # ---- phase E: attn(qt1); hi0 slots run proj(tt0), hi1 slots
            # ---- prelaunch proj(tt1)'s hi0 matmuls (et4/5 last: pst frees
            # ---- only after the final QK/exp pair)
            pu0 = proj_units(0, [(0, 3), (3, 3)])
            emit_attn(1, pu0 + [
                lambda: u_p1h0(0, pmm), lambda: u_p1h0(1, pmm),
                lambda: u_p1h0(2, pvt), lambda: u_p1h0(3, pvt),
                None, None,
                lambda: u_p1h0(4, pst), lambda: u_p1h0(5, pst)])
            # ---- phase F: proj(tt1) hi1 + stores, small last store ----
            ostA = cp.tile([128, 3 * TT], BF16, tag="ost1A")
            ostB = cp.tile([128, 2 * TT], BF16, tag="ost1B")
            ostC = cp.tile([128, TT], BF16, tag="ost1C")
            for col, et in enumerate(range(0, 3)):
                u_p1h1(et, ostA, col, split=False)
            u_store(1, 0, 3, ostA)
            for col, et in enumerate(range(3, 5)):
                u_p1h1(et, ostB, col, split=(et == 4))
            u_store(1, 3, 2, ostB)
            u_p1h1(5, ostC, 0, split=True)
            u_store(1, 5, 1, ostC)
    nc.compile()
    return nc


def _in_maps(x, W_attn, b_attn, W_proj, b_proj):
    bf = ml_dtypes.bfloat16
    xTn = x.reshape(T, C).T.astype(np.float32)  # [C, T]
    maps = []
    for core in range(NCORES):
        h0, h1 = HEAD_MAP[core]
        cols = []
        for part in range(3):  # q, k, v column groups of W_attn
            for h in (h0, h1):
                cols.extend(range(part * C + h * HS, part * C + (h + 1) * HS))
        wsel = W_attn[:, cols].astype(np.float32)                       # [C, 384]
        bqc = np.concatenate([b_attn[h0 * HS:(h0 + 1) * HS],
                              b_attn[h1 * HS:(h1 + 1) * HS]]
                             ).astype(np.float32).reshape(128, 1)
        wpc = np.concatenate(
            [W_proj[h0 * HS:(h0 + 1) * HS, :],
             np.zeros_like(W_proj[:HS]) if h1 == h0
             else W_proj[h1 * HS:(h1 + 1) * HS, :]], axis=0)            # [128, C]
        blob = np.empty((128, BLOBW), np.float32)
        for cc in range(NCC):
            r = slice(cc * 128, (cc + 1) * 128)
            seg = cc * SEG
            blob[:, seg:seg + 256] = wsel[r, 0:256]
            blob[:, seg + 256:seg + 384] = wsel[r, 256:384]
            blob[:, seg + 384:seg + 896] = xTn[r, 0:TT]
            blob[:, X1OFF + cc * TT:X1OFF + (cc + 1) * TT] = xTn[r, TT:T]
        blob[:, WPOFF:BLOBW] = wpc
        maps.append({
            "blob": np.ascontiguousarray(blob.astype(bf)),
            "bq": np.ascontiguousarray(bqc),
        })
    return maps


def kernel(x, W_attn, b_attn, W_proj, b_proj, _trace=False, _trace_kwargs=None):
    x = np.asarray(x, np.float32)
    W_attn = np.asarray(W_attn, np.float32)
    b_attn = np.asarray(b_attn, np.float32)
    W_proj = np.asarray(W_proj, np.float32)
    b_proj = np.asarray(b_proj, np.float32)

    if "nc" not in _CACHE:
        _CACHE["nc"] = _build_program()
    nc = _CACHE["nc"]

    maps = _in_maps(x, W_attn, b_attn, W_proj, b_proj)
    kw = {}
    if _trace:
        kw = dict(trace=True, **(_trace_kwargs or {}))
    br = run_bass_kernel_spmd(nc, maps, list(range(NCORES)), **kw)
    acc = np.zeros((C, T), np.float64)
    for core in range(NCORES):
        acc += br.results[core]["outT"].astype(np.float64)
    # host-side bias fold: b_v @ W_proj + b_proj (softmax rows sum to 1)
    bias = (b_attn[2 * C:].astype(np.float64) @ W_proj.astype(np.float64)
            + b_proj.astype(np.float64))
    out = np.ascontiguousarray((acc.T + bias[None, :]).astype(np.float32))
    out = out.reshape(1, T, C)
    _CACHE["last_results"] = br
    return out


# revision 20
# speedup vs baseline: 1.0224x; 1.0224x over previous
"""TRN2 Bass kernel for nn_CausalSelfAttention_4054449128214.

The reference returns out_s + stop_gradient(out_full - out_s), whose forward
value is exactly out_full — plain dense causal self-attention. So the kernel
computes: qkv = x@W_attn+b_attn, per-head causal softmax attention, y@W_proj+b_proj.

Sharding (8 cores, no collectives):
  Megatron head-parallel. Cores 0-3 own head pairs (0,1)..(6,7); cores 4-7 own
  heads 8..11 (run twice for SPMD shape-uniformity, second copy's W_proj rows
  zeroed). Each core computes its heads' Q/K columns, V^T directly via matmul
  (lhsT = x chunk, rhs = W_v), attention, and a partial row-sliced output
  projection; the host sums the 8 partials (the Megatron row-parallel
  all-reduce) and transposes back.

Perf structure:
  - all inputs packed into ONE dram blob, consumption-ordered; per-cc segments
    (wqk|wv|x0) so each DMA unlocks a full contraction-chunk of K/Q/V^T work
    (HWDGE issue is a serial 625ns/DMA resource — DMA count is precious).
  - all matmul operands bf16 (full PE rate, half DMA bytes, 2x DVE rate);
    fp32 PSUM accumulation.
  - K/Q/V^T accumulation groups interleaved per-cc so PE starts on chunk 0.
  - attention (Act-exp-limited) interleaved with the next phase's matmuls:
    attn(qt0) x qkv(tt1), attn(qt1) x proj(tt0).
  - bias algebra: b_k is softmax-invariant (dropped); b_v/b_proj fold into a
    host-side constant column vector (softmax rows sum to 1); only b_q in-kernel.
"""

import numpy as np
import ml_dtypes

import concourse.bacc as bacc
import concourse.mybir as mybir
import concourse.tile as tile
from concourse.bass_utils import run_bass_kernel_spmd

F32 = mybir.dt.float32
BF16 = mybir.dt.bfloat16

T = 1024          # sequence length
C = 768           # channels
NH = 12           # heads
HS = 64           # head size
NCORES = 8
TT = 512          # t-tile (matmul moving free dim)
NT = T // TT      # 2
NCC = C // 128    # 6 contraction chunks
NKC = T // 128    # 8 key chunks
SCALE = 1.0 / 8.0  # 1/sqrt(HS)

NDUMMY = 400              # PE p-state warmup matmuls (run during initial DMA wait)
SEG = 896                 # per-cc blob segment: wqk(256) | wv(128) | x0(512)
X1OFF = NCC * SEG         # 5376
WPOFF = X1OFF + NCC * TT  # 8448
BLOBW = WPOFF + C         # 9216

# core -> (head0, head1); cores 4-7 duplicate their head (2nd W_proj slice zeroed)
HEAD_MAP = [(0, 1), (2, 3), (4, 5), (6, 7), (8, 8), (9, 9), (10, 10), (11, 11)]

_CACHE: dict = {}


def _build_program():
    nc = bacc.Bacc("TRN2", target_bir_lowering=False, debug=False,
                   num_devices=NCORES)
    blob = nc.dram_tensor("blob", [128, BLOBW], BF16, kind="ExternalInput").ap()
    bq = nc.dram_tensor("bq", [128, 1], F32, kind="ExternalInput").ap()
    outT = nc.dram_tensor("outT", [C, T], BF16, kind="ExternalOutput").ap()

    with tile.TileContext(nc) as tc:
        with (
            tc.tile_pool(name="const", bufs=1) as cp,
            tc.tile_pool(name="e", bufs=8) as ep,
            tc.tile_pool(name="rb", bufs=4) as rbp,
            tc.tile_pool(name="pmm", bufs=2, space="PSUM") as pmm,
            tc.tile_pool(name="pst", bufs=2, space="PSUM") as pst,
            tc.tile_pool(name="pov", bufs=2, space="PSUM") as pov,
            tc.tile_pool(name="pvt", bufs=2, space="PSUM") as pvt,
        ):
            bsb = cp.tile([128, WPOFF], BF16, tag="bsb")
            wpt = cp.tile([64, 2 * C], BF16, tag="wpt")
            for cc in range(NCC):
                nc.sync.dma_start(out=bsb[:, cc * SEG:(cc + 1) * SEG],
                                  in_=blob[:, cc * SEG:(cc + 1) * SEG])
            nc.sync.dma_start(out=bsb[:, X1OFF:X1OFF + 3 * TT],
                              in_=blob[:, X1OFF:X1OFF + 3 * TT])
            nc.sync.dma_start(out=bsb[:, X1OFF + 3 * TT:WPOFF],
                              in_=blob[:, X1OFF + 3 * TT:WPOFF])
            nc.sync.dma_start(
                out=wpt[:].rearrange("p (h e) -> p h e", h=2),
                in_=blob[:, WPOFF:BLOBW].rearrange("(h p) e -> p h e", p=64))
            bq_sb = cp.tile([128, 1], F32, tag="bq")
            nc.gpsimd.dma_start(out=bq_sb[:], in_=bq)

            wqk = [bsb[:, cc * SEG:cc * SEG + 256] for cc in range(NCC)]
            wv = [bsb[:, cc * SEG + 256:cc * SEG + 384] for cc in range(NCC)]
            xts = [[bsb[:, cc * SEG + 384:(cc + 1) * SEG],
                    bsb[:, X1OFF + cc * TT:X1OFF + (cc + 1) * TT]]
                   for cc in range(NCC)]
            wps = [wpt[:, hi * C:(hi + 1) * C] for hi in range(2)]

            # V^T tiles: [128 keys, 65*2] with a ones column at 64 and 129
            vaug = [cp.tile([128, 130], BF16, tag=f"va{kc}", name=f"va{kc}")
                    for kc in range(NKC)]
            for kc in range(NKC):
                nc.vector.memset(vaug[kc][:, 64:130:65], 1.0)
            ones64 = cp.tile([1, 64], BF16, tag="ones64")
            nc.vector.memset(ones64[:], 1.0)
            # PE warmup: tiny matmuls during the initial DMA wait keep the
            # tensor engine's p-state ramping so real work runs at full clock
            pdum = pvt.tile([1, 1], F32, tag="pt", name="pdum")
            for _ in range(NDUMMY):
                nc.tensor.matmul(pdum[:], ones64[0:1, 0:1], ones64[0:1, 0:1],
                                 start=True, stop=True)

            # static causal masks for the DVE half of the mask work
            masks = []
            for kcr in range(4):
                m = cp.tile([128, TT], BF16, tag=f"mask{kcr}", name=f"mask{kcr}")
                nc.vector.memset(m[:], 1.0)
                nc.gpsimd.affine_select(
                    m[:], m[:], pattern=[[1, TT]],
                    compare_op=mybir.AluOpType.is_ge, fill=0.0,
                    base=-128 * kcr, channel_multiplier=-1)
                masks.append(m)

            qT = [None] * NT   # [128=(2h x 64hs), TT] bf16
            kT = [None] * NT
            yT = [[None] * NT for _ in range(2)]
            st8 = {}           # per-tt K/Q/V01 psum state

            def vcopy(kc, pt):
                nc.vector.tensor_copy(vaug[kc][:, 0:64], pt[:, 0:64])
                nc.vector.tensor_copy(vaug[kc][:, 65:129], pt[:, 64:128])

            def u_cc(tt, cc):
                # one contraction chunk of the K/Q/V0/V1 accumulations
                if cc == 0:
                    st8[tt] = (pmm.tile([128, TT], F32, tag="mm", name=f"psK{tt}"),
                               pmm.tile([128, TT], F32, tag="mm", name=f"psQ{tt}"),
                               pvt.tile([128, 128], F32, tag="pt", name=f"ptA{tt}"),
                               pvt.tile([128, 128], F32, tag="pt", name=f"ptB{tt}"))
                psK, psQ, ptA, ptB = st8[tt]
                st, sp = cc == 0, cc == NCC - 1
                nc.tensor.matmul(psK[:], wqk[cc][:, 128:256], xts[cc][tt],
                                 start=st, stop=sp)
                nc.tensor.matmul(psQ[:], wqk[cc][:, 0:128], xts[cc][tt],
                                 start=st, stop=sp)
                nc.tensor.matmul(ptA[:], xts[cc][tt][:, 0:128], wv[cc],
                                 start=st, stop=sp)
                nc.tensor.matmul(ptB[:], xts[cc][tt][:, 128:256], wv[cc],
                                 start=st, stop=sp)

            def u_fin(tt):
                psK, psQ, ptA, ptB = st8[tt]
                kT[tt] = cp.tile([128, TT], BF16, tag=f"k{tt}", name=f"k{tt}")
                nc.scalar.activation(
                    kT[tt][:], psK[:], mybir.ActivationFunctionType.Copy)
                qT[tt] = cp.tile([128, TT], BF16, tag=f"q{tt}", name=f"q{tt}")
                nc.vector.tensor_scalar_add(qT[tt][:], psQ[:], bq_sb[:, 0:1])
                vcopy(tt * 4 + 0, ptA)
                vcopy(tt * 4 + 1, ptB)

            def u_v23(tt, j):  # j in (2, 3)
                pt = pvt.tile([128, 128], F32, tag="pt")
                for cc in range(NCC):
                    nc.tensor.matmul(pt[:], xts[cc][tt][:, j * 128:(j + 1) * 128],
                                     wv[cc], start=(cc == 0), stop=(cc == NCC - 1))
                vcopy(tt * 4 + j, pt)

            def emit_attn(qt, fillers):
                fit = iter(fillers)
                # masked (diagonal) chunks first so the last chunk's chain is
                # QK->exp->PV with no mask step; order is math-irrelevant (sum)
                kcs = list(range(qt * 4, qt * 4 + 4)) + list(range(0, qt * 4))
                for hi in range(2):
                    po = pov.tile([65, TT], F32, tag="po")
                    for i, kc in enumerate(kcs):
                        last = i == len(kcs) - 1
                        ktile = kT[kc // 4]
                        kcol = (kc % 4) * 128
                        ps = pst.tile([128, TT], F32, tag="st")
                        nc.tensor.matmul(
                            ps[:], ktile[hi * 64:(hi + 1) * 64, kcol:kcol + 128],
                            qT[qt][hi * 64:(hi + 1) * 64, :],
                            start=True, stop=True)
                        e = ep.tile([128, TT], BF16, tag="e")
                        nc.scalar.activation(
                            e[:], ps[:], mybir.ActivationFunctionType.Exp,
                            scale=SCALE)
                        kcr = kc - qt * 4
                        if kcr >= 0:  # diagonal chunk: zero where tk > tq
                            if kcr % 2 == 0:
                                nc.gpsimd.affine_select(
                                    e[:], e[:], pattern=[[1, TT]],
                                    compare_op=mybir.AluOpType.is_ge, fill=0.0,
                                    base=-128 * kcr, channel_multiplier=-1)
                            else:
                                nc.vector.tensor_mul(e[:], e[:], masks[kcr][:])
                        va = vaug[kc][:, hi * 65:(hi + 1) * 65]
                        if not last:
                            nc.tensor.matmul(po[:], va, e[:],
                                             start=(i == 0), stop=False)
                        else:
                            # split the last PV by query halves so the two
                            # normalization chains pipeline
                            nc.tensor.matmul(po[:, 0:TT // 2], va,
                                             e[:, 0:TT // 2],
                                             start=False, stop=False)
                            nc.tensor.matmul(po[:, TT // 2:TT], va,
                                             e[:, TT // 2:TT],
                                             start=False, stop=True)
                        f = next(fit, None)
                        if f is not None:
                            f()
                    yT[hi][qt] = cp.tile([64, TT], BF16, tag=f"y{hi}_{qt}",
                                         name=f"y{hi}_{qt}")
                    # both recips first so the DVE in-order queue pipelines
                    # the two recip->broadcast->mul half-chains
                    rbcs = []
                    for c0 in (0, TT // 2):
                        c1 = c0 + TT // 2
                        rb = rbp.tile([1, TT // 2], BF16, tag="rb")
                        with nc.allow_low_precision(
                                reason="bf16 1/s is a uniform 0.4% softmax-"
                                       "scale wobble, within the 2e-2 gate"):
                            nc.vector.reciprocal(rb[0:1, :], po[64:65, c0:c1])
                        rbc = rbp.tile([64, TT // 2], BF16, tag="rbc")
                        nc.gpsimd.partition_broadcast(rbc[:], rb[0:1, :])
                        rbcs.append(rbc)
                    for rbc, c0 in zip(rbcs, (0, TT // 2)):
                        nc.vector.tensor_mul(yT[hi][qt][:, c0:c0 + TT // 2],
                                             po[0:64, c0:c0 + TT // 2], rbc[:])
                for f in fit:
                    f()

            def u_et(tt, et, ost, col, split=False):
                # one output-projection column group into ost[:, col*TT:...]
                pm = pmm.tile([128, TT], F32, tag="mm")
                for hi in range(2):
                    nc.tensor.matmul(
                        pm[:], wps[hi][:, et * 128:(et + 1) * 128],
                        yT[hi][tt][:], start=(hi == 0), stop=(hi == 1))
                dst = ost[:, col * TT:(col + 1) * TT]
                if split:  # halves on both engines so the last copy overlaps
                    nc.scalar.activation(dst[:, 0:TT // 2], pm[:, 0:TT // 2],
                                         mybir.ActivationFunctionType.Copy)
                    nc.vector.tensor_copy(dst[:, TT // 2:TT], pm[:, TT // 2:TT])
                elif et % 3 == 0:
                    nc.scalar.activation(
                        dst, pm[:], mybir.ActivationFunctionType.Copy)
                else:
                    nc.vector.tensor_copy(dst, pm[:])

            def u_store(tt, et0, ngrp, ost):
                nc.sync.dma_start(
                    out=outT[et0 * 128:(et0 + ngrp) * 128,
                             tt * TT:(tt + 1) * TT]
                    .rearrange("(g p) t -> p g t", p=128),
                    in_=ost[:].rearrange("p (g t) -> p g t", g=ngrp))

            def proj_units(tt, groups):
                units = []
                for et0, ngrp in groups:
                    ost = cp.tile([128, ngrp * TT], BF16,
                                  tag=f"ost{tt}_{et0}", name=f"ost{tt}_{et0}")
                    for i in range(ngrp):
                        units.append(lambda tt=tt, et=et0 + i, ost=ost, col=i:
                                     u_et(tt, et, ost, col))
                    units.append(lambda tt=tt, et0=et0, ngrp=ngrp, ost=ost:
                                 u_store(tt, et0, ngrp, ost))
                return units

            # --- split-phase proj for tt1: hi0 matmuls prelaunch into idle
            # --- psum banks during attn(1) hi1; hi1 matmuls+copies after
            pm1 = {}

            def u_p1h0(et, pool):
                pm1[et] = pool.tile([128, TT], F32, tag="mm" if pool is pmm
                                    else ("pt" if pool is pvt else "st"),
                                    name=f"pm1_{et}")
                nc.tensor.matmul(pm1[et][:], wps[0][:, et * 128:(et + 1) * 128],
                                 yT[0][1][:], start=True, stop=False)

            def u_p1h1(et, ost, col, split):
                nc.tensor.matmul(pm1[et][:], wps[1][:, et * 128:(et + 1) * 128],
                                 yT[1][1][:], start=False, stop=True)
                dst = ost[:, col * TT:(col + 1) * TT]
                if split:  # halves on both engines so the last copy overlaps
                    nc.scalar.activation(dst[:, 0:TT // 2], pm1[et][:, 0:TT // 2],
                                         mybir.ActivationFunctionType.Copy)
                    nc.vector.tensor_copy(dst[:, TT // 2:TT],
                                          pm1[et][:, TT // 2:TT])
                elif et % 2 == 0:
                    nc.scalar.activation(dst, pm1[et][:],
                                         mybir.ActivationFunctionType.Copy)
                else:
                    nc.vector.tensor_copy(dst, pm1[et][:])

            # ---- phase C: qkv+vaug for tt0, DMA-paced per-cc ----
            for cc in range(NCC):
                u_cc(0, cc)
            u_fin(0)
            u_v23(0, 2)
            u_v23(0, 3)
            # ---- phase D: attn(qt0) interleaved with qkv+vaug(tt1) ----
            emit_attn(0, [lambda cc=cc: u_cc(1, cc) for cc in range(NCC)]
                      + [lambda: u_fin(1), lambda: u_v23(1, 2)])
            u_v23(1, 3)
            # ---- phase E: attn(qt1); hi0 slots run proj(tt0), hi1 slots
            # ---- prelaunch proj(tt1)'s hi0 matmuls (et4/5 last: pst frees
            # ---- only after the final QK/exp pair)
            pu0 = proj_units(0, [(0, 3), (3, 3)])
            emit_attn(1, pu0 + [
                lambda: u_p1h0(0, pmm), lambda: u_p1h0(1, pmm),
                lambda: u_p1h0(2, pvt), lambda: u_p1h0(3, pvt)])
            # ---- phase F: proj(tt1) hi1 + stores, small last store ----
            ostA = cp.tile([128, 3 * TT], BF16, tag="ost1A")
            ostB = cp.tile([128, 2 * TT], BF16, tag="ost1B")
            ostC = cp.tile([128, TT], BF16, tag="ost1C")
            for col, et in enumerate(range(0, 3)):
                u_p1h1(et, ostA, col, split=False)
            u_store(1, 0, 3, ostA)
            u_p1h1(3, ostB, 0, split=False)
            u_et(1, 4, ostB, 1)
            u_store(1, 3, 2, ostB)
            u_et(1, 5, ostC, 0, split=True)
            u_store(1, 5, 1, ostC)
    nc.compile()
    return nc


def _in_maps(x, W_attn, b_attn, W_proj, b_proj):
    bf = ml_dtypes.bfloat16
    xTn = x.reshape(T, C).T.astype(np.float32)  # [C, T]
    maps = []
    for core in range(NCORES):
        h0, h1 = HEAD_MAP[core]
        cols = []
        for part in range(3):  # q, k, v column groups of W_attn
            for h in (h0, h1):
                cols.extend(range(part * C + h * HS, part * C + (h + 1) * HS))
        wsel = W_attn[:, cols].astype(np.float32)                       # [C, 384]
        bqc = np.concatenate([b_attn[h0 * HS:(h0 + 1) * HS],
                              b_attn[h1 * HS:(h1 + 1) * HS]]
                             ).astype(np.float32).reshape(128, 1)
        wpc = np.concatenate(
            [W_proj[h0 * HS:(h0 + 1) * HS, :],
             np.zeros_like(W_proj[:HS]) if h1 == h0
             else W_proj[h1 * HS:(h1 + 1) * HS, :]], axis=0)            # [128, C]
        blob = np.empty((128, BLOBW), np.float32)
        for cc in range(NCC):
            r = slice(cc * 128, (cc + 1) * 128)
            seg = cc * SEG
            blob[:, seg:seg + 256] = wsel[r, 0:256]
            blob[:, seg + 256:seg + 384] = wsel[r, 256:384]
            blob[:, seg + 384:seg + 896] = xTn[r, 0:TT]
            blob[:, X1OFF + cc * TT:X1OFF + (cc + 1) * TT] = xTn[r, TT:T]
        blob[:, WPOFF:BLOBW] = wpc
        maps.append({
            "blob": np.ascontiguousarray(blob.astype(bf)),
            "bq": np.ascontiguousarray(bqc),
        })
    return maps


def kernel(x, W_attn, b_attn, W_proj, b_proj, _trace=False, _trace_kwargs=None):
    x = np.asarray(x, np.float32)
    W_attn = np.asarray(W_attn, np.float32)
    b_attn = np.asarray(b_attn, np.float32)
    W_proj = np.asarray(W_proj, np.float32)
    b_proj = np.asarray(b_proj, np.float32)

    if "nc" not in _CACHE:
        _CACHE["nc"] = _build_program()
    nc = _CACHE["nc"]

    maps = _in_maps(x, W_attn, b_attn, W_proj, b_proj)
    kw = {}
    if _trace:
        kw = dict(trace=True, **(_trace_kwargs or {}))
    br = run_bass_kernel_spmd(nc, maps, list(range(NCORES)), **kw)
    acc = np.zeros((C, T), np.float64)
    for core in range(NCORES):
        acc += br.results[core]["outT"].astype(np.float64)
    # host-side bias fold: b_v @ W_proj + b_proj (softmax rows sum to 1)
    bias = (b_attn[2 * C:].astype(np.float64) @ W_proj.astype(np.float64)
            + b_proj.astype(np.float64))
    out = np.ascontiguousarray((acc.T + bias[None, :]).astype(np.float32))
    out = out.reshape(1, T, C)
    _CACHE["last_results"] = br
    return out


# revision 23
# speedup vs baseline: 1.1755x; 1.1498x over previous
"""TRN2 Bass kernel for nn_CausalSelfAttention_4054449128214.

The reference returns out_s + stop_gradient(out_full - out_s), whose forward
value is exactly out_full — plain dense causal self-attention. So the kernel
computes: qkv = x@W_attn+b_attn, per-head causal softmax attention, y@W_proj+b_proj.

Sharding (8 cores, no collectives):
  Megatron head-parallel. Cores 0-3 own head pairs (0,1)..(6,7); cores 4-7 own
  heads 8..11 (run twice for SPMD shape-uniformity, second copy's W_proj rows
  zeroed). Each core computes its heads' Q/K columns, V^T directly via matmul
  (lhsT = x chunk, rhs = W_v), attention, and a partial row-sliced output
  projection; the host sums the 8 partials (the Megatron row-parallel
  all-reduce) and transposes back.

Perf structure:
  - one consumption-ordered dram blob; per-cc segments (wqk|wv|x0) so each DMA
    unlocks a full contraction chunk of K/Q/V^T work (HWDGE issue is a serial
    625ns/DMA resource).
  - all matmul operands bf16 (full PE rate, half DMA bytes); fp32 PSUM.
  - PE p-state warmup: tiny matmuls during the initial DMA wait.
  - K/Q/V^T accumulation groups interleaved per-cc; attention interleaved
    with the next phase's matmuls.
  - attention query tiles split into 256-wide halves A/B with separate PSUM
    accumulators: causality lets half A skip the last two key chunks, so the
    A-half softmax/proj/store complete while B's chunks still run; the kernel
    tail is only the 256-wide B chain + 12 small proj matmuls + a tiny store.
  - bias algebra: b_k is softmax-invariant (dropped); b_v/b_proj fold into a
    host-side constant (softmax rows sum to 1); only b_q in-kernel.
"""

import numpy as np
import ml_dtypes

import concourse.bacc as bacc
import concourse.mybir as mybir
import concourse.tile as tile
from concourse.bass_utils import run_bass_kernel_spmd

F32 = mybir.dt.float32
BF16 = mybir.dt.bfloat16

T = 1024          # sequence length
C = 768           # channels
NH = 12           # heads
HS = 64           # head size
NCORES = 8
TT = 512          # t-tile (query tile)
H = TT // 2       # query half-tile
NT = T // TT      # 2
NCC = C // 128    # 6 contraction chunks
NKC = T // 128    # 8 key chunks
SCALE = 1.0 / 8.0  # 1/sqrt(HS)

NDUMMY = 400              # PE p-state warmup matmuls (run during initial DMA wait)
SEG = 896                 # per-cc blob segment: wqk(256) | wv(128) | x0(512)
X1OFF = NCC * SEG         # 5376
WPOFF = X1OFF + NCC * TT  # 8448
BLOBW = WPOFF + C         # 9216

# core -> (head0, head1); cores 4-7 duplicate their head (2nd W_proj slice zeroed)
HEAD_MAP = [(0, 1), (2, 3), (4, 5), (6, 7), (8, 8), (9, 9), (10, 10), (11, 11)]

_CACHE: dict = {}


def _build_program():
    nc = bacc.Bacc("TRN2", target_bir_lowering=False, debug=False,
                   num_devices=NCORES)
    blob = nc.dram_tensor("blob", [128, BLOBW], BF16, kind="ExternalInput").ap()
    bq = nc.dram_tensor("bq", [128, 1], F32, kind="ExternalInput").ap()
    outT = nc.dram_tensor("outT", [C, T], BF16, kind="ExternalOutput").ap()

    with tile.TileContext(nc) as tc:
        with (
            tc.tile_pool(name="const", bufs=1) as cp,
            tc.tile_pool(name="e", bufs=8) as ep,
            tc.tile_pool(name="rb", bufs=6) as rbp,
            tc.tile_pool(name="pmm", bufs=2, space="PSUM") as pmm,
            tc.tile_pool(name="pst", bufs=2, space="PSUM") as pst,
            tc.tile_pool(name="pov", bufs=2, space="PSUM") as pov,
            tc.tile_pool(name="pvt", bufs=2, space="PSUM") as pvt,
        ):
            bsb = cp.tile([128, WPOFF], BF16, tag="bsb")
            wpt = cp.tile([64, 2 * C], BF16, tag="wpt")
            for cc in range(NCC):
                nc.sync.dma_start(out=bsb[:, cc * SEG:(cc + 1) * SEG],
                                  in_=blob[:, cc * SEG:(cc + 1) * SEG])
            nc.sync.dma_start(out=bsb[:, X1OFF:X1OFF + 3 * TT],
                              in_=blob[:, X1OFF:X1OFF + 3 * TT])
            nc.sync.dma_start(out=bsb[:, X1OFF + 3 * TT:WPOFF],
                              in_=blob[:, X1OFF + 3 * TT:WPOFF])
            nc.sync.dma_start(
                out=wpt[:].rearrange("p (h e) -> p h e", h=2),
                in_=blob[:, WPOFF:BLOBW].rearrange("(h p) e -> p h e", p=64))
            bq_sb = cp.tile([128, 1], F32, tag="bq")
            nc.gpsimd.dma_start(out=bq_sb[:], in_=bq)

            wqk = [bsb[:, cc * SEG:cc * SEG + 256] for cc in range(NCC)]
            wv = [bsb[:, cc * SEG + 256:cc * SEG + 384] for cc in range(NCC)]
            xts = [[bsb[:, cc * SEG + 384:(cc + 1) * SEG],
                    bsb[:, X1OFF + cc * TT:X1OFF + (cc + 1) * TT]]
                   for cc in range(NCC)]
            wps = [wpt[:, hi * C:(hi + 1) * C] for hi in range(2)]

            ones64 = cp.tile([1, 64], BF16, tag="ones64")
            nc.vector.memset(ones64[:], 1.0)
            # PE warmup: tiny matmuls during the initial DMA wait keep the
            # tensor engine's p-state ramping so real work runs at full clock
            pdum = pvt.tile([1, 1], F32, tag="pt", name="pdum")
            for _ in range(NDUMMY):
                nc.tensor.matmul(pdum[:], ones64[0:1, 0:1], ones64[0:1, 0:1],
                                 start=True, stop=True)

            # V^T tiles: [128 keys, 65*2] with a ones column at 64 and 129
            vaug = [cp.tile([128, 130], BF16, tag=f"va{kc}", name=f"va{kc}")
                    for kc in range(NKC)]
            for kc in range(NKC):
                nc.vector.memset(vaug[kc][:, 64:130:65], 1.0)

            # static causal masks for the DVE half of the mask work
            masks = []
            for kcr in range(4):
                m = cp.tile([128, TT], BF16, tag=f"mask{kcr}", name=f"mask{kcr}")
                nc.vector.memset(m[:], 1.0)
                nc.gpsimd.affine_select(
                    m[:], m[:], pattern=[[1, TT]],
                    compare_op=mybir.AluOpType.is_ge, fill=0.0,
                    base=-128 * kcr, channel_multiplier=-1)
                masks.append(m)

            qT = [None] * NT   # [128=(2h x 64hs), TT] bf16
            kT = [None] * NT
            yT = [[None] * NT for _ in range(2)]
            st8 = {}           # per-tt K/Q/V01 psum state

            def vcopy(kc, pt):
                nc.vector.tensor_copy(vaug[kc][:, 0:64], pt[:, 0:64])
                nc.vector.tensor_copy(vaug[kc][:, 65:129], pt[:, 64:128])

            def u_cc(tt, cc):
                # one contraction chunk of the K/Q/V0/V1 accumulations
                if cc == 0:
                    st8[tt] = (pmm.tile([128, TT], F32, tag="mm", name=f"psK{tt}"),
                               pmm.tile([128, TT], F32, tag="mm", name=f"psQ{tt}"),
                               pvt.tile([128, 128], F32, tag="pt", name=f"ptA{tt}"),
                               pvt.tile([128, 128], F32, tag="pt", name=f"ptB{tt}"))
                psK, psQ, ptA, ptB = st8[tt]
                st, sp = cc == 0, cc == NCC - 1
                nc.tensor.matmul(psK[:], wqk[cc][:, 128:256], xts[cc][tt],
                                 start=st, stop=sp)
                nc.tensor.matmul(psQ[:], wqk[cc][:, 0:128], xts[cc][tt],
                                 start=st, stop=sp)
                nc.tensor.matmul(ptA[:], xts[cc][tt][:, 0:128], wv[cc],
                                 start=st, stop=sp)
                nc.tensor.matmul(ptB[:], xts[cc][tt][:, 128:256], wv[cc],
                                 start=st, stop=sp)

            def u_fin(tt):
                psK, psQ, ptA, ptB = st8[tt]
                kT[tt] = cp.tile([128, TT], BF16, tag=f"k{tt}", name=f"k{tt}")
                nc.scalar.activation(
                    kT[tt][:], psK[:], mybir.ActivationFunctionType.Copy)
                qT[tt] = cp.tile([128, TT], BF16, tag=f"q{tt}", name=f"q{tt}")
                nc.vector.tensor_scalar_add(qT[tt][:], psQ[:], bq_sb[:, 0:1])
                vcopy(tt * 4 + 0, ptA)
                vcopy(tt * 4 + 1, ptB)

            def u_v23(tt, j):  # j in (2, 3)
                pt = pvt.tile([128, 128], F32, tag="pt")
                for cc in range(NCC):
                    nc.tensor.matmul(pt[:], xts[cc][tt][:, j * 128:(j + 1) * 128],
                                     wv[cc], start=(cc == 0), stop=(cc == NCC - 1))
                vcopy(tt * 4 + j, pt)

            def chain(po, hi, qt, c0):
                # 256-wide softmax denominator: recip -> broadcast -> scale
                rb = rbp.tile([1, H], BF16, tag="rb")
                with nc.allow_low_precision(
                        reason="bf16 1/s is a uniform 0.4% softmax-scale "
                               "wobble, within the 2e-2 gate"):
                    nc.vector.reciprocal(rb[0:1, :], po[64:65, 0:H])
                rbc = rbp.tile([64, H], BF16, tag="rbc")
                nc.gpsimd.partition_broadcast(rbc[:], rb[0:1, :])
                nc.vector.tensor_mul(yT[hi][qt][:, c0:c0 + H],
                                     po[0:64, 0:H], rbc[:])

            def emit_attn(qt, fillers):
                """Causal attention for query tile qt, queries split into
                halves A ([0,256)) and B ([256,512)) with separate PSUM
                accumulators. Half A needs no keys from the last two chunks,
                so its chain can fire two chunks early."""
                fit = iter(fillers)
                diagA = [qt * 4, qt * 4 + 1]
                below = list(range(0, qt * 4))
                halfB = [qt * 4 + 2, qt * 4 + 3]
                plan = ([(kc, False) for kc in diagA + below]
                        + [(kc, True) for kc in halfB])
                pos_a_last = len(diagA + below) - 1
                for hi in range(2):
                    if yT[hi][qt] is None:
                        yT[hi][qt] = cp.tile([64, TT], BF16, tag=f"y{hi}_{qt}",
                                             name=f"y{hi}_{qt}")
                    poA = pov.tile([65, TT], F32, tag="po", name=f"poA{hi}_{qt}")
                    poB = pov.tile([65, TT], F32, tag="po", name=f"poB{hi}_{qt}")
                    for i, (kc, bonly) in enumerate(plan):
                        ktile = kT[kc // 4]
                        kcol = (kc % 4) * 128
                        kcr = kc - qt * 4
                        ps = pst.tile([128, TT], F32, tag="st")
                        qs = qT[qt][hi * 64:(hi + 1) * 64, :]
                        if not bonly:
                            nc.tensor.matmul(
                                ps[:], ktile[hi * 64:(hi + 1) * 64,
                                             kcol:kcol + 128],
                                qs, start=True, stop=True)
                            e = ep.tile([128, TT], BF16, tag="e")
                            nc.scalar.activation(
                                e[:], ps[:], mybir.ActivationFunctionType.Exp,
                                scale=SCALE)
                            if kcr >= 0:  # diagonal in half A; B fully live
                                if kcr == 0:
                                    nc.gpsimd.affine_select(
                                        e[:, 0:H], e[:, 0:H],
                                        pattern=[[1, H]],
                                        compare_op=mybir.AluOpType.is_ge,
                                        fill=0.0, base=0, channel_multiplier=-1)
                                else:
                                    nc.vector.tensor_mul(
                                        e[:, 0:H], e[:, 0:H],
                                        masks[1][:, 0:H])
                            nc.tensor.matmul(
                                poA[:, 0:H], vaug[kc][:, hi * 65:(hi + 1) * 65],
                                e[:, 0:H], start=(i == 0),
                                stop=(i == pos_a_last))
                            nc.tensor.matmul(
                                poB[:, 0:H], vaug[kc][:, hi * 65:(hi + 1) * 65],
                                e[:, H:TT], start=(i == 0), stop=False)
                        else:
                            # half-B-only chunk: 256-wide scores/exp/mask/PV
                            nc.tensor.matmul(
                                ps[:, 0:H], ktile[hi * 64:(hi + 1) * 64,
                                                  kcol:kcol + 128],
                                qs[:, H:TT], start=True, stop=True)
                            e = ep.tile([128, H], BF16, tag="eh", name=f"eh{i}")
                            nc.scalar.activation(
                                e[:], ps[:, 0:H],
                                mybir.ActivationFunctionType.Exp, scale=SCALE)
                            if kcr == 2:
                                nc.gpsimd.affine_select(
                                    e[:], e[:], pattern=[[1, H]],
                                    compare_op=mybir.AluOpType.is_ge,
                                    fill=0.0, base=0, channel_multiplier=-1)
                            else:
                                nc.vector.tensor_mul(e[:], e[:],
                                                     masks[1][:, 0:H])
                            nc.tensor.matmul(
                                poB[:, 0:H], vaug[kc][:, hi * 65:(hi + 1) * 65],
                                e[:], start=False,
                                stop=(i == len(plan) - 1))
                        if i == pos_a_last:
                            chain(poA, hi, qt, 0)
                        f = next(fit, None)
                        if f is not None:
                            f()
                    chain(poB, hi, qt, H)
                for f in fit:
                    f()

            def u_et(tt, et, ost, col, split=False):
                # full-width output-projection column group
                pm = pmm.tile([128, TT], F32, tag="mm")
                for hi in range(2):
                    nc.tensor.matmul(
                        pm[:], wps[hi][:, et * 128:(et + 1) * 128],
                        yT[hi][tt][:], start=(hi == 0), stop=(hi == 1))
                dst = ost[:, col * TT:(col + 1) * TT]
                if split:
                    nc.scalar.activation(dst[:, 0:H], pm[:, 0:H],
                                         mybir.ActivationFunctionType.Copy)
                    nc.vector.tensor_copy(dst[:, H:TT], pm[:, H:TT])
                elif et % 3 == 0:
                    nc.scalar.activation(
                        dst, pm[:], mybir.ActivationFunctionType.Copy)
                else:
                    nc.vector.tensor_copy(dst, pm[:])

            def u_store(tt, et0, ngrp, ost):
                nc.sync.dma_start(
                    out=outT[et0 * 128:(et0 + ngrp) * 128,
                             tt * TT:(tt + 1) * TT]
                    .rearrange("(g p) t -> p g t", p=128),
                    in_=ost[:].rearrange("p (g t) -> p g t", g=ngrp))

            def proj_units(tt, groups):
                units = []
                for et0, ngrp in groups:
                    ost = cp.tile([128, ngrp * TT], BF16,
                                  tag=f"ost{tt}_{et0}", name=f"ost{tt}_{et0}")
                    for i in range(ngrp):
                        units.append(lambda tt=tt, et=et0 + i, ost=ost, col=i:
                                     u_et(tt, et, ost, col))
                    units.append(lambda tt=tt, et0=et0, ngrp=ngrp, ost=ost:
                                 u_store(tt, et0, ngrp, ost))
                return units

            # --- half-width proj for tt1 (half = 0 for A, 1 for B) ---
            pm1 = {}

            def u_ph0(et, half, pool):
                pm1[(et, half)] = pool.tile(
                    [128, TT], F32, tag="mm" if pool is pmm else "pt",
                    name=f"pm1_{et}_{half}")
                nc.tensor.matmul(
                    pm1[(et, half)][:, 0:H],
                    wps[0][:, et * 128:(et + 1) * 128],
                    yT[0][1][:, half * H:(half + 1) * H], start=True, stop=False)

            def u_ph1(et, half, ost, col, split=False):
                key = (et, half)
                if key not in pm1:
                    pm1[key] = pmm.tile([128, TT], F32, tag="mm",
                                        name=f"pm1_{et}_{half}")
                    nc.tensor.matmul(
                        pm1[key][:, 0:H], wps[0][:, et * 128:(et + 1) * 128],
                        yT[0][1][:, half * H:(half + 1) * H],
                        start=True, stop=False)
                nc.tensor.matmul(
                    pm1[key][:, 0:H], wps[1][:, et * 128:(et + 1) * 128],
                    yT[1][1][:, half * H:(half + 1) * H], start=False, stop=True)
                dst = ost[:, col * H:(col + 1) * H]
                if split:
                    nc.scalar.activation(dst[:, 0:H // 2], pm1[key][:, 0:H // 2],
                                         mybir.ActivationFunctionType.Copy)
                    nc.vector.tensor_copy(dst[:, H // 2:H],
                                          pm1[key][:, H // 2:H])
                elif et % 2 == 0:
                    nc.scalar.activation(dst, pm1[key][:, 0:H],
                                         mybir.ActivationFunctionType.Copy)
                else:
                    nc.vector.tensor_copy(dst, pm1[key][:, 0:H])

            def u_store_h(half, et0, ngrp, ost):
                c0 = TT + half * H
                nc.sync.dma_start(
                    out=outT[et0 * 128:(et0 + ngrp) * 128, c0:c0 + H]
                    .rearrange("(g p) t -> p g t", p=128),
                    in_=ost[:].rearrange("p (g t) -> p g t", g=ngrp))

            # ---- phase C: qkv+vaug for tt0, DMA-paced per-cc ----
            for cc in range(NCC):
                u_cc(0, cc)
            u_fin(0)
            u_v23(0, 2)
            u_v23(0, 3)
            # ---- phase D: attn(qt0) interleaved with qkv+vaug(tt1) ----
            emit_attn(0, [lambda cc=cc: u_cc(1, cc) for cc in range(NCC)]
                      + [lambda: u_fin(1), lambda: u_v23(1, 2)])
            u_v23(1, 3)
            # ---- phase E: attn(qt1); hi0 slots run proj(tt0), hi1 slots
            # ---- prelaunch the A-half of proj(tt1)'s hi0 matmuls ----
            pu0 = proj_units(0, [(0, 3), (3, 3)])
            ostA = cp.tile([128, 3 * H], BF16, tag="ost1A")
            ostA2 = cp.tile([128, 3 * H], BF16, tag="ost1A2")
            ostB = cp.tile([128, 3 * H], BF16, tag="ost1B")
            ostC = cp.tile([128, 2 * H], BF16, tag="ost1C")
            ostD = cp.tile([128, H], BF16, tag="ost1D")

            def tail_a():
                # proj of tt1 half A: runs while B's last chunks + chain go
                for col, et in enumerate(range(0, 3)):
                    u_ph1(et, 0, ostA, col)
                u_store_h(0, 0, 3, ostA)
                for col, et in enumerate(range(3, 6)):
                    u_ph1(et, 0, ostA2, col)
                u_store_h(0, 3, 3, ostA2)

            # tail_a lands in the post-loop drain: its PE matmuls then fill
            # the chain-B latency window instead of delaying poB's chunks
            emit_attn(1, pu0 + [
                lambda: u_ph0(0, 0, pmm), lambda: u_ph0(1, 0, pmm),
                lambda: u_ph0(2, 0, pvt), lambda: u_ph0(3, 0, pvt),
                None, None, None, None, tail_a])
            # ---- phase F: proj of tt1 half B (the only true tail) ----
            for col, et in enumerate(range(0, 3)):
                u_ph1(et, 1, ostB, col)
            u_store_h(1, 0, 3, ostB)
            for col, et in enumerate(range(3, 5)):
                u_ph1(et, 1, ostC, col)
            u_store_h(1, 3, 2, ostC)
            u_ph1(5, 1, ostD, 0, split=True)
            u_store_h(1, 5, 1, ostD)
    nc.compile()
    return nc


def _in_maps(x, W_attn, b_attn, W_proj, b_proj):
    bf = ml_dtypes.bfloat16
    xTn = x.reshape(T, C).T.astype(np.float32)  # [C, T]
    maps = []
    for core in range(NCORES):
        h0, h1 = HEAD_MAP[core]
        cols = []
        for part in range(3):  # q, k, v column groups of W_attn
            for h in (h0, h1):
                cols.extend(range(part * C + h * HS, part * C + (h + 1) * HS))
        wsel = W_attn[:, cols].astype(np.float32)                       # [C, 384]
        bqc = np.concatenate([b_attn[h0 * HS:(h0 + 1) * HS],
                              b_attn[h1 * HS:(h1 + 1) * HS]]
                             ).astype(np.float32).reshape(128, 1)
        wpc = np.concatenate(
            [W_proj[h0 * HS:(h0 + 1) * HS, :],
             np.zeros_like(W_proj[:HS]) if h1 == h0
             else W_proj[h1 * HS:(h1 + 1) * HS, :]], axis=0)            # [128, C]
        blob = np.empty((128, BLOBW), np.float32)
        for cc in range(NCC):
            r = slice(cc * 128, (cc + 1) * 128)
            seg = cc * SEG
            blob[:, seg:seg + 256] = wsel[r, 0:256]
            blob[:, seg + 256:seg + 384] = wsel[r, 256:384]
            blob[:, seg + 384:seg + 896] = xTn[r, 0:TT]
            blob[:, X1OFF + cc * TT:X1OFF + (cc + 1) * TT] = xTn[r, TT:T]
        blob[:, WPOFF:BLOBW] = wpc
        maps.append({
            "blob": np.ascontiguousarray(blob.astype(bf)),
            "bq": np.ascontiguousarray(bqc),
        })
    return maps


def kernel(x, W_attn, b_attn, W_proj, b_proj, _trace=False, _trace_kwargs=None):
    x = np.asarray(x, np.float32)
    W_attn = np.asarray(W_attn, np.float32)
    b_attn = np.asarray(b_attn, np.float32)
    W_proj = np.asarray(W_proj, np.float32)
    b_proj = np.asarray(b_proj, np.float32)

    if "nc" not in _CACHE:
        _CACHE["nc"] = _build_program()
    nc = _CACHE["nc"]

    maps = _in_maps(x, W_attn, b_attn, W_proj, b_proj)
    kw = {}
    if _trace:
        kw = dict(trace=True, **(_trace_kwargs or {}))
    br = run_bass_kernel_spmd(nc, maps, list(range(NCORES)), **kw)
    acc = np.zeros((C, T), np.float64)
    for core in range(NCORES):
        acc += br.results[core]["outT"].astype(np.float64)
    # host-side bias fold: b_v @ W_proj + b_proj (softmax rows sum to 1)
    bias = (b_attn[2 * C:].astype(np.float64) @ W_proj.astype(np.float64)
            + b_proj.astype(np.float64))
    out = np.ascontiguousarray((acc.T + bias[None, :]).astype(np.float32))
    out = out.reshape(1, T, C)
    _CACHE["last_results"] = br
    return out
